# revision 1
# baseline (speedup 1.0000x reference)
"""Trainium2 Bass kernel for nn_CapsuleLayer (dynamic routing).

Problem:  u_hat = einsum('bri,crio->cbro', x, W);  3 routing iterations
          (softmax over R, weighted sum, squash, agreement update).
Shapes:   x [256, 1152, 8] f32, W [10, 1152, 8, 16] f32 ->
          out [10, 256, 1, 1, 16] f32.

Strategy (8 NeuronCores, data-parallel over batch, B_loc = 32/core):
  * never materialize u_hat (189 MB) in HBM;
  * s-sums   : PE matmuls, K = 128-row r-blocks, i via 8 accumulating
               matmuls, y = softmax-weights * x built on DVE/GPSIMD;
  * agreement: PE matmuls with block-diagonal v stationaries streaming a
               (c,o)-partition W copy (streamed from HBM per iteration),
               then fused multiply + i-tree, L accumulated in fp32;
  * softmax  : ACT exp with accumulated Z, weights transposed to r-block
               partitions with PE transposes.
All arithmetic fp32 (bf16 anywhere in the logit path measurably breaks
the output tolerance: ~1e-2 rel err per bf16-rounded component).
"""

import sys
from contextlib import ExitStack

import numpy as np

sys.path.insert(0, "/opt/trn_rl_repo")

import concourse.bacc as bacc
import concourse.bass as bass
import concourse.mybir as mybir
import concourse.tile as tile
from concourse.bass_utils import run_bass_kernel_spmd

F32 = mybir.dt.float32
F16 = mybir.dt.float16
MUL = mybir.AluOpType.mult
ADD = mybir.AluOpType.add

B, R, I, C, O = 256, 1152, 8, 10, 16
NC = 8
BL = B // NC          # 32 batch per core
Q = R // 128          # 9 r-blocks of 128
CO = C * O            # 160
RI = R * I            # 9216
EPS = 1e-7
GCH = 1024            # g-matmul free-dim chunk (elements of (r,i))
NG = RI // GCH        # 9 chunks
W3 = 3                # (c,b) waves


def build_nc(debug=False):
    nc = bacc.Bacc("TRN2", target_bir_lowering=False, debug=debug)

    xtr_d = nc.declare_dram_parameter("xtr", [128, Q, I, BL], F32, isOutput=False)
    wfr_d = nc.declare_dram_parameter("wfr", [128, Q, I, CO], F32, isOutput=False)
    wt_d = nc.declare_dram_parameter("wt", [4, 16, 3, RI], F16, isOutput=False)
    xrep_d = nc.declare_dram_parameter("xrep", [128, RI], F16, isOutput=False)
    ident_d = nc.declare_dram_parameter("ident", [128, 128], F32, isOutput=False)
    out_d = nc.declare_dram_parameter("out", [C, O, BL], F32, isOutput=True)

    with tile.TileContext(nc) as tc, ExitStack() as ctx:
        res = ctx.enter_context(tc.tile_pool(name="res", bufs=1))
        cwp = ctx.enter_context(tc.tile_pool(name="cwp", bufs=2))
        yp = ctx.enter_context(tc.tile_pool(name="yp", bufs=2))
        wtp = ctx.enter_context(tc.tile_pool(name="wtp", bufs=2))
        gmp = ctx.enter_context(tc.tile_pool(name="gmp", bufs=2))
        trp = ctx.enter_context(tc.tile_pool(name="trp", bufs=2))
        smp = ctx.enter_context(tc.tile_pool(name="smp", bufs=1))
        psS = ctx.enter_context(
            tc.tile_pool(name="psS", bufs=1, space=bass.MemorySpace.PSUM)
        )
        psG = ctx.enter_context(
            tc.tile_pool(name="psG", bufs=2, space=bass.MemorySpace.PSUM)
        )
        psT = ctx.enter_context(
            tc.tile_pool(name="psT", bufs=2, space=bass.MemorySpace.PSUM)
        )
        psN = ctx.enter_context(
            tc.tile_pool(name="psN", bufs=1, space=bass.MemorySpace.PSUM)
        )

        # ---- resident tensors -------------------------------------
        xtr = res.tile([128, Q, I, BL], F32)
        wfr = res.tile([128, Q, I, CO], F32)
        xrep = res.tile([128, RI], F16)
        ident = res.tile([128, 128], F32)
        L = res.tile([128, W3, R], F32)
        cwT = res.tile([128, Q, W3, 128], F32)
        Z = res.tile([128, W3], F32)
        Zi = res.tile([128, W3], F32)
        vblk = res.tile([128, 3, 32], F16)   # v[c, b, o] staged at rows 32P+o
        ones16 = res.tile([16, 1], F32)
        v_sb = res.tile([16, C, BL], F32)    # squash output, [o, c, b]

        nc.sync.dma_start(xtr[:], xtr_d[:])
        nc.sync.dma_start(wfr[:], wfr_d[:])
        nc.sync.dma_start(xrep[:], xrep_d[:])
        nc.sync.dma_start(ident[:], ident_d[:])
        nc.vector.memset(L[:], 0.0)
        nc.vector.memset(ones16[:], 1.0)

        # ---------------------------------------------------------------
        def build_y(c):
            """y_c[rr, q, i, b] = cw[c, b, 128q+rr] * x[b, 128q+rr, i].

            cw comes from cwT (r-block partitions); (c,b) column index in
            cwT is p = 32k + b, where class c = 4w + k (w=2: c = 8 + k).
            """
            w = min(c // 4, 2)
            k = c - 4 * w
            y_c = yp.tile([128, Q, I, BL], F32, tag="y")
            cw_src = (
                cwT[:, :, w, 32 * k : 32 * k + 32]
                .unsqueeze(2)
                .broadcast_to([128, Q, I, BL])
            )
            eng = nc.vector if c % 2 == 0 else nc.gpsimd
            eng.tensor_tensor(y_c, xtr[:], cw_src, MUL)
            return y_c

        def s_pass(it):
            """sps[o, c, b] = sum_{r,i} rhs_c[r, i, b] * W[c, r, i, o]."""
            sps = psS.tile([16, C, BL], F32, tag="sps")
            for c in range(C):
                y_c = None if it == 0 else build_y(c)
                for q in range(Q):
                    for i in range(I):
                        rhs = xtr[:, q, i, :] if it == 0 else y_c[:, q, i, :]
                        nc.tensor.matmul(
                            sps[:, c, :],
                            wfr[:, q, i, c * O : (c + 1) * O],
                            rhs,
                            start=(q == 0 and i == 0),
                            stop=(q == Q - 1 and i == I - 1),
                        )
            return sps

        # ---------------------------------------------------------------
        def squash(sps, it):
            """v_sb = squash(s) over o;  it==0 folds the uniform 1/R weight."""
            sq = smp.tile([16, C * BL], F32, tag="sq")
            nc.scalar.activation(
                sq[:],
                sps[:].rearrange("o c b -> o (c b)"),
                mybir.ActivationFunctionType.Square,
            )
            snps = psN.tile([1, C * BL], F32, tag="snps")
            nc.tensor.matmul(snps[:], ones16[:], sq[:], start=True, stop=True)
            sn = smp.tile([1, C * BL], F32, tag="sn")
            if it == 0:
                nc.vector.tensor_scalar_mul(sn[:], snps[:], 1.0 / (R * R))
            else:
                nc.vector.tensor_copy(sn[:], snps[:])
            u1 = smp.tile([1, C * BL], F32, tag="u1")
            u2 = smp.tile([1, C * BL], F32, tag="u2")
            u3 = smp.tile([1, C * BL], F32, tag="u3")
            u4 = smp.tile([1, C * BL], F32, tag="u4")
            f = smp.tile([1, C * BL], F32, tag="f")
            nc.vector.tensor_scalar_add(u1[:], sn[:], EPS)
            nc.scalar.activation(u2[:], u1[:], mybir.ActivationFunctionType.Sqrt)
            nc.vector.tensor_scalar_add(u3[:], sn[:], 1.0)
            nc.vector.tensor_tensor(u4[:], u2[:], u3[:], MUL)
            nc.vector.reciprocal(u1[:], u4[:])
            nc.vector.tensor_tensor(f[:], sn[:], u1[:], MUL)
            if it == 0:
                nc.vector.tensor_scalar_mul(f[:], f[:], 1.0 / R)
            frep = smp.tile([16, C * BL], F32, tag="frep")
            nc.gpsimd.partition_broadcast(frep[:], f[:])
            nc.vector.tensor_tensor(
                v_sb[:].rearrange("o c b -> o (c b)"),
                sps[:].rearrange("o c b -> o (c b)"),
                frep[:],
                MUL,
            )

        def fill_vblk():
            """Stage v as g-matmul stationaries (fp16): slot cc of strip P
            holds class 2P+cc; slot 2 of strip P holds class 8+P (P<2)."""
            for c in range(C):
                P, slot = (c // 2, c % 2) if c < 8 else (c - 8, 2)
                nc.vector.tensor_copy(
                    vblk[32 * P : 32 * P + 16, slot, :], v_sb[:, c, :]
                )

        # ---------------------------------------------------------------
        def agreement():
            """L[p, w, r] += sum_o v[c,b,o]*W[c,r,i,o] (*) x[b,r,i], sum_i."""
            for n0 in range(NG):
                off = n0 * GCH
                wt_t = wtp.tile([128, 3, GCH], F16, tag="wt")
                for P in range(4):
                    nc.sync.dma_start(
                        wt_t[32 * P : 32 * P + 16, :, :],
                        wt_d[P, :, :, off : off + GCH],
                    )
                for w in range(W3):
                    gps = psG.tile([128, GCH], F32, tag="gps")
                    npart = 128 if w < 2 else 64
                    for k in range(4 if w < 2 else 2):
                        c = 4 * w + k
                        P, slot = (c // 2, c % 2) if c < 8 else (c - 8, 2)
                        base = 32 * P
                        for sub in range(0, GCH, 512):
                            nc.tensor.matmul(
                                gps[32 * k : 32 * k + 32, sub : sub + 512],
                                vblk[base : base + 16, slot, :],
                                wt_t[base : base + 16, slot, sub : sub + 512],
                                start=True,
                                stop=True,
                                tile_position=(base, 32 * k),
                            )
                    # fused drain-multiply:  gm = gps * xrep   (fp32, DVE)
                    gm = gmp.tile([128, GCH], F32, tag="gm")
                    nc.vector.tensor_tensor(
                        gm[:npart, :],
                        gps[:npart, :],
                        xrep[:npart, off : off + GCH],
                        MUL,
                    )
                    # i-reduction tree (8 -> 4 -> 2 -> 1) on GPSIMD
                    nr = GCH // I
                    l1 = trp.tile([128, GCH // 2], F32, tag="l1")
                    l2 = trp.tile([128, GCH // 4], F32, tag="l2")
                    a = trp.tile([128, GCH // 8], F32, tag="a")
                    gmv = gm.rearrange("p (r i) -> p r i", i=I)
                    l1v = l1.rearrange("p (r i) -> p r i", i=4)
                    l2v = l2.rearrange("p (r i) -> p r i", i=2)
                    nc.gpsimd.tensor_tensor(
                        l1v[:npart], gmv[:npart, :, 0:4], gmv[:npart, :, 4:8], ADD
                    )
                    nc.gpsimd.tensor_tensor(
                        l2v[:npart], l1v[:npart, :, 0:2], l1v[:npart, :, 2:4], ADD
                    )
                    nc.gpsimd.tensor_tensor(
                        a[:npart], l2v[:npart, :, 0], l2v[:npart, :, 1], ADD
                    )
                    r0 = off // I
                    nc.vector.tensor_tensor(
                        L[:npart, w, r0 : r0 + nr],
                        L[:npart, w, r0 : r0 + nr],
                        a[:npart],
                        ADD,
                    )

        # ---------------------------------------------------------------
        def softmax_transpose():
            """cw = softmax_r(L) per (c,b); write transposed into cwT."""
            for w in range(W3):
                cwv = cwp.tile([128, R], F32, tag="cw")
                nc.scalar.activation(
                    cwv[:],
                    L[:, w, :],
                    mybir.ActivationFunctionType.Exp,
                    accum_out=Z[:, w : w + 1],
                )
                nc.vector.reciprocal(Zi[:, w : w + 1], Z[:, w : w + 1])
                nc.vector.tensor_scalar_mul(cwv[:], cwv[:], Zi[:, w : w + 1])
                for q in range(Q):
                    tps = psT.tile([128, 128], F32, tag="tps")
                    nc.tensor.transpose(
                        tps[:], cwv[:, 128 * q : 128 * (q + 1)], ident[:]
                    )
                    nc.scalar.copy(cwT[:, q, w, :], tps[:])

        # =========================== flow ==============================
        for it in range(3):
            if it > 0:
                softmax_transpose()
            sps = s_pass(it)
            squash(sps, it)
            if it < 2:
                fill_vblk()
                agreement()

        nc.sync.dma_start(out_d[:].rearrange("c o b -> o c b"), v_sb[:])

    nc.compile()
    return nc


# =================== host-side prep / entry point =====================

def _prep_shared(W):
    """Per-problem constant tensors (replicated on every core)."""
    W = np.ascontiguousarray(W, np.float32)
    # wfr[rr, q, i, 16c+o] = W[c, 128q+rr, i, o]
    wfr = np.ascontiguousarray(
        W.reshape(C, Q, 128, I, O).transpose(2, 1, 3, 0, 4).reshape(128, Q, I, CO)
    )
    # wt[P, o, slot, 8r+i]: slot cc<2 -> W[2P+cc]; slot 2 -> W[8+P] (P<2).
    wt = np.zeros((4, 16, 3, RI), np.float16)
    for P in range(4):
        for cc in range(2):
            wt[P, :, cc, :] = W[2 * P + cc].transpose(2, 0, 1).reshape(O, RI)
    for P in range(2):
        wt[P, :, 2, :] = W[8 + P].transpose(2, 0, 1).reshape(O, RI)
    ident = np.eye(128, dtype=np.float32)
    return wfr, wt, ident


def _prep_core(x_shard):
    """Per-core tensors for one 32-batch shard: xtr and xrep."""
    xs = np.ascontiguousarray(x_shard, np.float32)       # [32, 1152, 8]
    xtr = np.ascontiguousarray(
        xs.reshape(BL, Q, 128, I).transpose(2, 1, 3, 0)
    )                                                     # [128, Q, I, 32]
    flat = xs.reshape(BL, RI)                             # [b, 8r+i]
    xrep = np.ascontiguousarray(
        flat[np.arange(128) % BL].astype(np.float16)
    )                                                     # [128, RI]
    return xtr, xrep


_NC_CACHE = {}


def kernel(x, W):
    x = np.asarray(x, np.float32)
    W = np.asarray(W, np.float32)
    if "nc" not in _NC_CACHE:
        _NC_CACHE["nc"] = build_nc()
    nc = _NC_CACHE["nc"]

    wfr, wt, ident = _prep_shared(W)
    in_maps = []
    for m in range(NC):
        xtr, xrep = _prep_core(x[m * BL : (m + 1) * BL])
        in_maps.append(
            {"xtr": xtr, "wfr": wfr, "wt": wt, "xrep": xrep, "ident": ident}
        )

    res = run_bass_kernel_spmd(nc, in_maps, list(range(NC)))
    out = np.empty((C, B, 1, 1, O), np.float32)
    for m in range(NC):
        o = res.results[m]["out"]                         # [C, O, BL]
        out[:, m * BL : (m + 1) * BL, 0, 0, :] = np.asarray(o).transpose(0, 2, 1)
    return out


if __name__ == "__main__":
    d = np.load("/root/problem/ref_data.npz")
    got = kernel(d["x"], d["W"])
    exp = d["expected"]
    err = np.abs(got - exp).max() / np.abs(exp).max()
    print("Relative error:", err)



# revision 11
# speedup vs baseline: 1.3846x; 1.3846x over previous
"""Trainium2 Bass kernel for nn_CapsuleLayer (dynamic routing), v2.

Problem:  u_hat = einsum('bri,crio->cbro', x, W);  3 routing iterations
          (softmax over R, weighted sum, squash, agreement update).
Shapes:   x [256, 1152, 8] f32, W [10, 1152, 8, 16] f32 ->
          out [10, 256, 1, 1, 16] f32.

v2 design (8 NeuronCores, data-parallel over batch, B_loc = 32/core):
  * all matmul operands fp16 (PSUM accumulation fp32, logits L fp32);
  * it0 s-pass: 72 wide matmuls, stationary = x r-block [128,32],
    moving = W r-block over all 10 classes [128,160], out [b,(c,o)];
  * it1/2 s-pass: per class, stationary = W[rr,16], moving = y[rr,32];
  * agreement: classes spread over 4 PE row-strips (c = 4w+k -> strip k,
    wave w) so the four per-chunk g-matmuls run concurrently; fused
    multiply with b-replicated x and a fully contiguous i-reduction tree
    (wt/xrep stored i-outermost per 128-r chunk);
  * softmax normalization deferred: y = exp(L-max)*x unnormalized, the
    1/Z factor (Z via ones-stationary matmuls on transposed cw) is folded
    into squash; cw transposes per 128-r block on the PE;
  * per-wave interleaving: agreement wave w -> exp/transpose/y-build ->
    s-matmuls of wave w classes, so tensor work streams without stalls;
  * all weight tensors resident in SBUF (no streaming DMA inside loops),
    startup DMAs chunked so compute starts early.
"""

import sys
from contextlib import ExitStack

import numpy as np

sys.path.insert(0, "/opt/trn_rl_repo")

import concourse.bacc as bacc
import concourse.bass as bass
import concourse.mybir as mybir
import concourse.tile as tile
from concourse.bass_utils import run_bass_kernel_spmd

F32 = mybir.dt.float32
F16 = mybir.dt.float16
MUL = mybir.AluOpType.mult
ADD = mybir.AluOpType.add
AX = mybir.ActivationFunctionType

B, R, I, C, O = 256, 1152, 8, 10, 16
NC = 8
BL = B // NC          # 32 batch per core
Q = R // 128          # 9 r-blocks of 128 (also agreement chunks)
CO = C * O            # 160
RI = R * I            # 9216
EPS = 1e-7
W3 = 3                # waves: classes c = 4w+k, k<4 (w<2), k<2 (w=2)


def _nk(w):
    return 4 if w < 2 else 2


def build_nc(debug=False):
    nc = bacc.Bacc("TRN2", target_bir_lowering=False, debug=debug)

    xtr_d = nc.declare_dram_parameter("xtr", [128, Q, I, BL], F16, isOutput=False)
    wfr_d = nc.declare_dram_parameter("wfr", [128, Q, I, CO], F16, isOutput=False)
    wta_d = nc.declare_dram_parameter("wta", [128, 2, Q, I, 128], F16, isOutput=False)
    wtb_d = nc.declare_dram_parameter("wtb", [64, Q, I, 128], F16, isOutput=False)
    xrep_d = nc.declare_dram_parameter("xrep", [128, Q, I, 128], F16, isOutput=False)
    ident_d = nc.declare_dram_parameter("ident", [128, 128], F32, isOutput=False)
    out_d = nc.declare_dram_parameter("out", [C, O, BL], F32, isOutput=True)

    with tile.TileContext(nc) as tc, ExitStack() as ctx:
        res = ctx.enter_context(tc.tile_pool(name="res", bufs=1))
        yp = ctx.enter_context(tc.tile_pool(name="yp", bufs=2))
        cvp = ctx.enter_context(tc.tile_pool(name="cvp", bufs=2))
        gmp = ctx.enter_context(tc.tile_pool(name="gmp", bufs=2))
        trp = ctx.enter_context(tc.tile_pool(name="trp", bufs=2))
        smp = ctx.enter_context(tc.tile_pool(name="smp", bufs=1))
        psG = ctx.enter_context(
            tc.tile_pool(name="psG", bufs=2, space=bass.MemorySpace.PSUM)
        )
        psT = ctx.enter_context(
            tc.tile_pool(name="psT", bufs=2, space=bass.MemorySpace.PSUM)
        )
        psS = ctx.enter_context(
            tc.tile_pool(name="psS", bufs=1, space=bass.MemorySpace.PSUM)
        )
        psZ = ctx.enter_context(
            tc.tile_pool(name="psZ", bufs=1, space=bass.MemorySpace.PSUM)
        )

        # ---- resident tensors -------------------------------------
        xtr = res.tile([128, Q, I, BL], F16)
        wfr = res.tile([128, Q, I, CO], F16)
        wta = res.tile([128, 2, Q, I, 128], F16)   # rows 32k+o: class 4w+k, w<2
        wtb = res.tile([64, Q, I, 128], F16)       # rows 32k+o: class 8+k
        xrep = res.tile([128, Q, I, 128], F16)     # rows 32k+b: x[b, 128q+rl, i]
        ident = res.tile([128, 128], F32)
        L = res.tile([128, W3, R], F32)            # logits, rows 32k+b
        cwT = res.tile([128, Q, W3, 128], F16)     # cw transposed: [rr, q, w, (k,b)]
        vblk = res.tile([128, W3, 32], F16)        # rows 32k+o: v[4w+k, b, o]
        v_sb = res.tile([16, C * BL], F32)         # squash output [o, (c,b)]
        ones16 = res.tile([16, 1], F16)
        ones128 = res.tile([128, 1], F16)

        # startup DMAs: gate it0 on xtr+wfr; stream the rest per-q chunk
        nc.sync.dma_start(xtr[:], xtr_d[:])
        nc.sync.dma_start(wfr[:], wfr_d[:])
        for q in range(Q):
            nc.sync.dma_start(wta[:, :, q, :, :], wta_d[:, :, q, :, :])
            nc.sync.dma_start(wtb[:, q, :, :], wtb_d[:, q, :, :])
            nc.sync.dma_start(xrep[:, q, :, :], xrep_d[:, q, :, :])
        nc.sync.dma_start(ident[:], ident_d[:])
        nc.vector.memset(ones16[:], 1.0)
        nc.vector.memset(ones128[:], 1.0)
        nc.vector.memset(L[:], 0.0)

        sps = psS.tile([32, C * BL], F32, tag="sps")   # it0 view [32,160] / it view [16,320]
        zps = psZ.tile([1, W3 * 128], F32, tag="zps")

        # ---------------------------------------------------------------
        def s_pass0():
            """sps[b, (c,o)] = sum_{q,rr,i} x[b,128q+rr,i] * W[c,128q+rr,i,o]."""
            out = sps[:, 0:CO]
            for q in range(Q):
                for i in range(I):
                    nc.tensor.matmul(
                        out,
                        xtr[:, q, i, :],
                        wfr[:, q, i, :],
                        start=(q == 0 and i == 0),
                        stop=(q == Q - 1 and i == I - 1),
                    )

        # =========================== flow ==============================

        # ---- iteration 0: uniform-weight s-pass + squash -------------
        s_pass0()

        # squash0 (layout [32 b, (c,o)]), folds the uniform 1/R weight
        s0 = smp.tile([32, C, O], F32, tag="s0")
        nc.scalar.copy(s0[:].rearrange("p c o -> p (c o)"), sps[:, 0:CO])
        sq0 = smp.tile([32, C, O], F32, tag="sq0")
        nc.scalar.activation(
            sq0[:].rearrange("p c o -> p (c o)"),
            s0[:].rearrange("p c o -> p (c o)"),
            AX.Square,
        )
        t1 = smp.tile([32, C, 8], F32, tag="t1")
        t2 = smp.tile([32, C, 4], F32, tag="t2")
        sn0 = smp.tile([32, C, 2], F32, tag="sn0pre")
        snf = smp.tile([32, C], F32, tag="sn0")
        nc.vector.tensor_tensor(t1[:], sq0[:, :, 0:8], sq0[:, :, 8:16], ADD)
        nc.vector.tensor_tensor(t2[:], t1[:, :, 0:4], t1[:, :, 4:8], ADD)
        nc.vector.tensor_tensor(sn0[:], t2[:, :, 0:2], t2[:, :, 2:4], ADD)
        nc.vector.tensor_tensor(
            snf[:].unsqueeze(2), sn0[:, :, 0:1], sn0[:, :, 1:2], ADD
        )
        nc.vector.tensor_scalar_mul(snf[:], snf[:], 1.0 / (R * R))
        u1 = smp.tile([32, C], F32, tag="u1s")
        u2 = smp.tile([32, C], F32, tag="u2s")
        u3 = smp.tile([32, C], F32, tag="u3s")
        f0 = smp.tile([32, C], F32, tag="f0s")
        nc.vector.tensor_scalar_add(u1[:], snf[:], EPS)
        nc.scalar.activation(u2[:], u1[:], AX.Sqrt)
        nc.vector.tensor_scalar_add(u3[:], snf[:], 1.0)
        nc.vector.tensor_tensor(u1[:], u2[:], u3[:], MUL)
        nc.vector.reciprocal(u2[:], u1[:])
        nc.vector.tensor_tensor(f0[:], snf[:], u2[:], MUL)
        nc.vector.tensor_scalar_mul(f0[:], f0[:], 1.0 / R)
        v0 = smp.tile([32, C, O], F32, tag="v0")
        nc.vector.tensor_tensor(
            v0[:], s0[:], f0[:].unsqueeze(2).broadcast_to([32, C, O]), MUL
        )
        # stage v0 -> vblk: pack per wave into [32 b, (k,o)] cols 32k+o,
        # then one PE transpose lands rows 32k+o = vblk layout directly.
        for w in range(W3):
            v0w = smp.tile([32, 4, 32], F32, tag="v0w")
            nc.vector.memset(v0w[:], 0.0)
            for k in range(_nk(w)):
                nc.vector.tensor_copy(v0w[:, k, 0:16], v0[:, 4 * w + k, :])
            tv = psT.tile([128, 128], F32, tag="tps")
            nc.tensor.transpose(
                tv[:, 0:32],
                v0w[:].rearrange("p k o -> p (k o)"),
                ident[0:32, 0:32],
            )
            nc.scalar.copy(vblk[:, w, :], tv[:, 0:32])

        # ---- boundaries: agreement(it) + softmax + y + s-pass(it+1) ----
        def agree_wave(it, w):
            nk = _nk(w)
            npart = 32 * nk
            for n0 in range(Q):
                gps = psG.tile([128, 1024], F32, tag="gps")
                for k in range(nk):
                    if w < 2:
                        wsrc = wta[32 * k : 32 * k + 16, w, n0, :, :]
                    else:
                        wsrc = wtb[32 * k : 32 * k + 16, n0, :, :]
                    for sub in range(2):
                        nc.tensor.matmul(
                            gps[32 * k : 32 * k + 32, 512 * sub : 512 * sub + 512],
                            vblk[32 * k : 32 * k + 16, w, :],
                            wsrc.rearrange("p i r -> p (i r)")[
                                :, 512 * sub : 512 * sub + 512
                            ],
                            start=True,
                            stop=True,
                            tile_position=(32 * k, 32 * k),
                        )
                gm = gmp.tile([128, 1024], F16, tag="gm")
                nc.vector.tensor_tensor(
                    gm[0:npart, :],
                    gps[0:npart, :],
                    xrep[0:npart, n0, :, :].rearrange("p i r -> p (i r)"),
                    MUL,
                )
                l1 = trp.tile([128, 512], F16, tag="l1")
                l2 = trp.tile([128, 256], F16, tag="l2")
                nc.gpsimd.tensor_tensor(l1[0:npart, :], gm[0:npart, 0:512], gm[0:npart, 512:1024], ADD)
                nc.gpsimd.tensor_tensor(l2[0:npart, :], l1[0:npart, 0:256], l1[0:npart, 256:512], ADD)
                rsl = slice(128 * n0, 128 * n0 + 128)
                if it == 0:
                    nc.gpsimd.tensor_tensor(
                        L[0:npart, w, rsl], l2[0:npart, 0:128], l2[0:npart, 128:256], ADD
                    )
                else:
                    dl = trp.tile([128, 128], F32, tag="dl")
                    nc.gpsimd.tensor_tensor(dl[0:npart, :], l2[0:npart, 0:128], l2[0:npart, 128:256], ADD)
                    nc.gpsimd.tensor_tensor(L[0:npart, w, rsl], L[0:npart, w, rsl], dl[0:npart, :], ADD)

        def softmax_y_wave(w):
            """exp(L - rowmax) -> cwT (transposed, fp16), Z matmuls, y build."""
            nk = _nk(w)
            npart = 32 * nk
            m = smp.tile([128, 1], F32, tag="rmax")
            nc.vector.reduce_max(m[0:npart, :], L[0:npart, w, :], axis=mybir.AxisListType.X)
            negm = smp.tile([128, 1], F32, tag="negm")
            nc.vector.tensor_scalar_mul(negm[0:npart, :], m[0:npart, :], -1.0)
            cwv = cvp.tile([128, R], F32, tag="cwv")
            nc.scalar.activation(
                cwv[0:npart, :], L[0:npart, w, :], AX.Exp, bias=negm[0:npart, :]
            )
            for q in range(Q):
                tps = psT.tile([128, 128], F32, tag="tps")
                nc.tensor.transpose(
                    tps[:, 0:npart],
                    cwv[0:npart, 128 * q : 128 * q + 128],
                    ident[0:npart, 0:npart],
                )
                nc.scalar.copy(cwT[:, q, w, 0:npart], tps[:, 0:npart])
                nc.tensor.matmul(
                    zps[:, 128 * w : 128 * w + npart],
                    ones128[:],
                    cwT[:, q, w, 0:npart],
                    start=(q == 0),
                    stop=(q == Q - 1),
                )
            y = yp.tile([128, Q, 4, I, BL], F16, tag="y")
            for q in range(Q):
                eng = nc.vector if q % 2 == 0 else nc.gpsimd
                eng.tensor_tensor(
                    y[:, q, 0:nk, :, :],
                    xtr[:, q, :, :].unsqueeze(1).broadcast_to([128, nk, I, BL]),
                    cwT[:, q, w, 0 : 32 * nk]
                    .rearrange("p (k b) -> p k b", b=32)
                    .unsqueeze(2)
                    .broadcast_to([128, nk, I, BL]),
                    MUL,
                )
            return y

        def s_mm_wave(w, y):
            nk = _nk(w)
            spsv = sps[0:16, :].rearrange("p (c b) -> p c b", b=BL)
            for k in range(nk):
                c = 4 * w + k
                for q in range(Q):
                    for i in range(I):
                        nc.tensor.matmul(
                            spsv[:, c, :],
                            wfr[:, q, i, c * O : (c + 1) * O],
                            y[:, q, k, i, :],
                            start=(q == 0 and i == 0),
                            stop=(q == Q - 1 and i == I - 1),
                        )

        def squash_it(last):
            """squash with deferred softmax normalization (Z folded in)."""
            zsb = smp.tile([1, C * BL], F32, tag="zsb")
            nc.scalar.copy(zsb[:], zps[:, 0 : C * BL])
            zi = smp.tile([1, C * BL], F32, tag="zi")
            nc.vector.reciprocal(zi[:], zsb[:])
            zirep = smp.tile([16, C * BL], F32, tag="zirep")
            nc.gpsimd.partition_broadcast(zirep[:], zi[:])
            s = smp.tile([16, C * BL], F32, tag="s")
            nc.vector.tensor_tensor(s[:], sps[0:16, :], zirep[:], MUL)
            sq = smp.tile([16, C * BL], F16, tag="sq")
            nc.scalar.activation(sq[:], s[:], AX.Square)
            snu = smp.tile([1, C * BL], F32, tag="snu")
            for j in range(3):  # 320 = 3 chunks of <=128 free
                lo = 128 * j
                hi = min(C * BL, lo + 128)
                tps = psT.tile([128, 128], F32, tag="tps")
                nc.tensor.matmul(
                    tps[0:1, 0 : hi - lo], ones16[:], sq[:, lo:hi],
                    start=True, stop=True,
                )
                nc.scalar.copy(snu[:, lo:hi], tps[0:1, 0 : hi - lo])
            sn = snu
            a1 = smp.tile([1, C * BL], F32, tag="a1")
            a2 = smp.tile([1, C * BL], F32, tag="a2")
            a3 = smp.tile([1, C * BL], F32, tag="a3")
            f = smp.tile([1, C * BL], F32, tag="f")
            nc.vector.tensor_scalar_add(a1[:], sn[:], EPS)
            nc.scalar.activation(a2[:], a1[:], AX.Sqrt)
            nc.vector.tensor_scalar_add(a3[:], sn[:], 1.0)
            nc.vector.tensor_tensor(a1[:], a2[:], a3[:], MUL)
            nc.vector.reciprocal(a2[:], a1[:])
            nc.vector.tensor_tensor(f[:], sn[:], a2[:], MUL)
            frep = smp.tile([16, C * BL], F32, tag="frep")
            nc.gpsimd.partition_broadcast(frep[:], f[:])
            nc.vector.tensor_tensor(v_sb[:], s[:], frep[:], MUL)
            if not last:
                vv = v_sb[:].rearrange("p (c b) -> p c b", b=BL)
                v16 = smp.tile([16, C, BL], F16, tag="v16")
                nc.vector.tensor_copy(v16[:], vv[:])
                for c in range(C):
                    w, k = c // 4, c % 4
                    nc.scalar.copy(vblk[32 * k : 32 * k + 16, w, :], v16[:, c, :])

        # boundary 0: agreement(it0) + softmax + s-pass(it1)
        for w in range(W3):
            agree_wave(0, w)
            y = softmax_y_wave(w)
            s_mm_wave(w, y)
        squash_it(last=False)

        # boundary 1: agreement(it1) + softmax + s-pass(it2)
        for w in range(W3):
            agree_wave(1, w)
            y = softmax_y_wave(w)
            s_mm_wave(w, y)
        squash_it(last=True)

        nc.sync.dma_start(
            out_d[:].rearrange("c o b -> o c b"),
            v_sb[:].rearrange("p (c b) -> p c b", b=BL),
        )

    nc.compile()
    return nc


# =================== host-side prep / entry point =====================

def _prep_shared(W):
    """Per-problem constant tensors (replicated on every core)."""
    W = np.ascontiguousarray(W, np.float32)
    # wfr[rr, q, i, 16c+o] = W[c, 128q+rr, i, o]
    wfr = np.ascontiguousarray(
        W.reshape(C, Q, 128, I, O).transpose(2, 1, 3, 0, 4).reshape(128, Q, I, CO)
    ).astype(np.float16)
    # wta[32k+o, w, q, i, rl] = W[4w+k, 128q+rl, i, o]  (w<2)
    Wr = W.reshape(C, Q, 128, I, O)                      # [c, q, rl, i, o]
    wta = np.zeros((4, 32, 2, Q, I, 128), np.float16)
    for w in range(2):
        for k in range(4):
            wta[k, 0:16, w] = Wr[4 * w + k].transpose(3, 0, 2, 1)  # [o, q, i, rl]
    wta = wta.reshape(128, 2, Q, I, 128)
    # wtb[32k+o, q, i, rl] = W[8+k, 128q+rl, i, o]
    wtb = np.zeros((2, 32, Q, I, 128), np.float16)
    for k in range(2):
        wtb[k, 0:16] = Wr[8 + k].transpose(3, 0, 2, 1)
    wtb = wtb.reshape(64, Q, I, 128)
    ident = np.eye(128, dtype=np.float32)
    return wfr, wta, wtb, ident


def _prep_core(x_shard):
    """Per-core tensors for one 32-batch shard: xtr and xrep."""
    xs = np.ascontiguousarray(x_shard, np.float32)       # [32, 1152, 8]
    xq = xs.reshape(BL, Q, 128, I)                       # [b, q, rl, i]
    xtr = np.ascontiguousarray(xq.transpose(2, 1, 3, 0)).astype(np.float16)
    # xrep[32k+b, q, i, rl] = x[b, 128q+rl, i]
    xr = np.ascontiguousarray(xq.transpose(0, 1, 3, 2)).astype(np.float16)  # [b,q,i,rl]
    xrep = np.broadcast_to(xr[None], (4, BL, Q, I, 128)).reshape(128, Q, I, 128)
    xrep = np.ascontiguousarray(xrep)
    return xtr, xrep


def build_inmaps(x, W):
    wfr, wta, wtb, ident = _prep_shared(W)
    in_maps = []
    for m in range(NC):
        xtr, xrep = _prep_core(x[m * BL : (m + 1) * BL])
        in_maps.append(
            {"xtr": xtr, "wfr": wfr, "wta": wta, "wtb": wtb,
             "xrep": xrep, "ident": ident}
        )
    return in_maps


_NC_CACHE = {}


def kernel(x, W):
    x = np.asarray(x, np.float32)
    W = np.asarray(W, np.float32)
    if "nc" not in _NC_CACHE:
        _NC_CACHE["nc"] = build_nc()
    nc = _NC_CACHE["nc"]

    in_maps = build_inmaps(x, W)
    res = run_bass_kernel_spmd(nc, in_maps, list(range(NC)))
    out = np.empty((C, B, 1, 1, O), np.float32)
    for m in range(NC):
        o = res.results[m]["out"]                         # [C, O, BL]
        out[:, m * BL : (m + 1) * BL, 0, 0, :] = np.asarray(o).transpose(0, 2, 1)
    return out


if __name__ == "__main__":
    d = np.load("/root/problem/ref_data.npz")
    got = kernel(d["x"], d["W"])
    exp = d["expected"]
    err = np.abs(got - exp).max() / np.abs(exp).max()
    print("Relative error:", err)
